# revision 1
# baseline (speedup 1.0000x reference)
"""Multi-head attention (B=4, G=2048, C=1024, H=16) on 8 TRN2 NeuronCores.

Sharding: (batch x head-half). Core c handles batch c//2 and an 8-head
slice (c%2). Each core computes its heads' q/k/v projections, full
softmax attention, and a partial output projection over its 512
channels; the host sums core pairs and adds the bias.

Device kernel (Bass/Tile, all matmuls as float32r):
  - qT/kT in [o, g] layout straight from the projection matmuls (x is
    pre-transposed on the host, so no on-device transposes anywhere).
  - scores are computed transposed ([k, q]); softmax needs no max
    subtraction (scores are small by construction) and the denominator
    comes for free from a ones-column appended to v.
  - exp on ScalarE fused with the 1/sqrt(d) scale.
"""

from contextlib import ExitStack

import numpy as np

import concourse.bass as bass
import concourse.tile as tile
from concourse import mybir
from concourse.bass_utils import run_bass_kernel_spmd
from concourse.vector_clock import ScopedClock, VectorClock
from concourse.tile_sem_assignment import N_PROCS

F32 = mybir.dt.float32
F32R = mybir.dt.float32r

B, G, C, H = 4, 2048, 1024, 16
N_CORES = 8
H_LOC = H // 2
O_LOC = H_LOC * 64


class SplitDrainTileContext(tile.TileContext):
    """Tail drain limited to one sync wait per instruction.

    This environment's walrus rejects >1 sync wait per instruction, so
    wait on each outstanding proc tick with its own NOP first and emit
    the drain bare.
    """

    def _drain_and_barrier(self, tick_clock, wait_clock):
        g = tick_clock.global_clock
        for p in range(N_PROCS):
            if g[p] > 0:
                nop = self.nc.sync.nop(nofuse=True)
                partial = VectorClock([g[q] if q == p else 0 for q in range(N_PROCS)])
                wait_clock.add_sem_waits(nop.ins, ScopedClock({None: partial}))
        self.nc.sync.drain()
        self.nc.all_engine_barrier()
        assert self.sems is not None
        popped = self.nc._tile_sem_poison_stack.pop()
        assert popped is self._sem_poison
        self.nc.clear_and_free_semaphores(list(self.sems.allocated().values()))
        self.nc.all_engine_barrier()


def split_multi_waits(nc):
    """Hoist extra sync waits onto NOPs before each offending instruction
    (this walrus accepts at most one sync wait per instruction)."""
    n_split = 0
    for f in nc.m.functions:
        for bb in f.blocks:
            insts = bb.instructions
            out = []
            for inst in insts:
                si = inst.sync_info
                waits = list(si.on_wait) if si and si.on_wait else []
                if len(waits) > 1:
                    for w in waits[:-1]:
                        nop = mybir.InstNoOp(
                            name=f"{inst.name}_w{n_split}",
                            engine=inst.engine,
                            ins=[],
                            outs=[],
                            sync_info=mybir.SyncInfo(on_wait=[w], on_update=[]),
                        )
                        out.append(nop)
                        n_split += 1
                    inst.sync_info = mybir.SyncInfo(
                        on_wait=[waits[-1]],
                        on_update=list(si.on_update) if si.on_update else [],
                    )
                out.append(inst)
            if len(out) != len(insts):
                bb.instructions[:] = out
    return n_split


def build_program():
    D = 64
    scale = D ** -0.5
    CC = C // 128
    OC = O_LOC // 128
    GC = G // 128
    KC = G // 128

    nc = bass.Bass()
    xT = nc.declare_dram_parameter("xT", [C, G], F32, isOutput=False)
    wqT = nc.declare_dram_parameter("wqT", [C, O_LOC], F32, isOutput=False)
    wkT = nc.declare_dram_parameter("wkT", [C, O_LOC], F32, isOutput=False)
    wvT = nc.declare_dram_parameter("wvT", [C, O_LOC], F32, isOutput=False)
    wpT = nc.declare_dram_parameter("wpT", [O_LOC, C], F32, isOutput=False)
    out_p = nc.declare_dram_parameter("out_p", [G, C], F32, isOutput=True)

    rcp_dram = nc.dram_tensor("rcp_scratch", [H_LOC, G], F32)

    with SplitDrainTileContext(nc) as tc, ExitStack() as ctx:
        persist = ctx.enter_context(tc.tile_pool(name="persist", bufs=1))
        qT_t = [persist.tile([128, G], F32R, name=f"qT{t}", tag=f"qT{t}") for t in range(OC)]
        kT_t = [persist.tile([128, G], F32R, name=f"kT{t}", tag=f"kT{t}") for t in range(OC)]
        v_sb = persist.tile([128, GC, H_LOC, 65], F32R, name="v_sb", tag="v_sb")

        # ---------------- phase 1: QKV projections ----------------
        with tc.tile_pool(name="ph1_w", bufs=1) as wpool, \
             tc.tile_pool(name="ph1_x", bufs=1) as xpool, \
             tc.tile_pool(name="ph1_ps", bufs=4, space="PSUM") as ps1:
            wq_sb = wpool.tile([128, CC, O_LOC], F32R, name="wq_sb", tag="wq")
            wk_sb = wpool.tile([128, CC, O_LOC], F32R, name="wk_sb", tag="wk")
            wv_sb = wpool.tile([128, CC, O_LOC], F32R, name="wv_sb", tag="wv")
            nc.sync.dma_start(out=wq_sb[:], in_=wqT.rearrange("(cc p) o -> p cc o", p=128).bitcast(F32R))
            nc.sync.dma_start(out=wk_sb[:], in_=wkT.rearrange("(cc p) o -> p cc o", p=128).bitcast(F32R))
            nc.sync.dma_start(out=wv_sb[:], in_=wvT.rearrange("(cc p) o -> p cc o", p=128).bitcast(F32R))
            ones_t = wpool.tile([128, GC, H_LOC, 1], F32, name="ones_t", tag="ones")
            nc.vector.memset(ones_t[:], 1.0)
            nc.vector.tensor_copy(out=v_sb[:, :, :, 64:65], in_=ones_t[:])

            GH = G // 2
            for gh in range(2):
                xh = xpool.tile([128, CC, GH], F32R, name="xh", tag="xh")
                nc.sync.dma_start(
                    out=xh[:],
                    in_=xT[:, gh * GH:(gh + 1) * GH].rearrange("(cc p) g -> p cc g", p=128).bitcast(F32R),
                )
                for w_sb, dst in ((wq_sb, qT_t), (wk_sb, kT_t)):
                    for oc in range(OC):
                        for z in range(GH // 512):
                            ps = ps1.tile([128, 512], F32, name="ps_qk", tag="ps_qk")
                            for cc in range(CC):
                                nc.tensor.matmul(
                                    ps[:],
                                    w_sb[:, cc, oc * 128:(oc + 1) * 128],
                                    xh[:, cc, z * 512:(z + 1) * 512],
                                    start=(cc == 0), stop=(cc == CC - 1),
                                )
                            nc.scalar.copy(
                                out=dst[oc][:, gh * GH + z * 512: gh * GH + (z + 1) * 512],
                                in_=ps[:],
                            )
                for gc8 in range(GH // 128):
                    gc = gh * (GH // 128) + gc8
                    ps = ps1.tile([128, O_LOC], F32, name="ps_v", tag="ps_v")
                    for cc in range(CC):
                        nc.tensor.matmul(
                            ps[:],
                            xh[:, cc, gc8 * 128:(gc8 + 1) * 128],
                            wv_sb[:, cc, :],
                            start=(cc == 0), stop=(cc == CC - 1),
                        )
                    nc.vector.tensor_copy(out=v_sb[:, gc, :, 0:64], in_=ps[:])

        # ---------------- phase 2+3 persistent SBUF ----------------
        p23 = ctx.enter_context(tc.tile_pool(name="p23", bufs=1))
        oT_t = [p23.tile([128, G], F32R, name=f"oT{t}", tag=f"oT{t}") for t in range(OC)]
        wp_sb = p23.tile([128, O_LOC // 128, C], F32R, name="wp_sb", tag="wp")
        nc.sync.dma_start(out=wp_sb[:], in_=wpT.rearrange("(ct p) o -> p ct o", p=128).bitcast(F32R))

        # ---------------- phase 2: attention ----------------
        with tc.tile_pool(name="ph2_exp", bufs=3) as epool, \
             tc.tile_pool(name="ph2_den", bufs=2) as dpool, \
             tc.tile_pool(name="ph2_bc", bufs=2) as bcpool, \
             tc.tile_pool(name="ph2_sc", bufs=2, space="PSUM") as scps, \
             tc.tile_pool(name="ph2_av", bufs=1, space="PSUM") as avps:
            for h in range(H_LOC):
                t, base = h // 2, (h % 2) * 64
                av = avps.tile([65, G], F32, name="av", tag="av")
                for kc in range(KC):
                    for qh in range(G // 1024):
                        sc = scps.tile([128, 1024], F32, name="sc", tag="sc")
                        for z in range(2):
                            nc.tensor.matmul(
                                sc[:, z * 512:(z + 1) * 512],
                                kT_t[t][base:base + D, kc * 128:(kc + 1) * 128],
                                qT_t[t][base:base + D,
                                        qh * 1024 + z * 512: qh * 1024 + (z + 1) * 512],
                                start=True, stop=True,
                            )
                        ex = epool.tile([128, 1024], F32R, name="ex", tag="ex")
                        nc.scalar.activation(
                            out=ex[:], in_=sc[:],
                            func=mybir.ActivationFunctionType.Exp, scale=scale,
                        )
                        for z in range(2):
                            nc.tensor.matmul(
                                av[:, qh * 1024 + z * 512: qh * 1024 + (z + 1) * 512],
                                v_sb[:, kc, h, :],
                                ex[:, z * 512:(z + 1) * 512],
                                start=(kc == 0), stop=(kc == KC - 1),
                            )
                den_row = dpool.tile([1, G], F32, name="den_row", tag="den_row")
                nc.vector.tensor_copy(out=den_row[:], in_=av[64:65, :])
                den_h = dpool.tile([128, G // 128], F32, name="den_h", tag="den_h")
                nc.sync.dma_start(
                    out=den_h[:],
                    in_=bass.AP(tensor=den_row.tensor, offset=den_row.offset,
                                ap=[[1, 1], [G // 128, 128], [1, G // 128]]),
                )
                nc.vector.reciprocal(out=den_h[:], in_=den_h[:])
                nc.sync.dma_start(out=rcp_dram[h, :], in_=den_h[:])
                bc = bcpool.tile([64, G], F32, name="bc", tag="bc")
                row = rcp_dram[h, :]
                nc.sync.dma_start(
                    out=bc[:],
                    in_=bass.AP(tensor=row.tensor, offset=row.offset,
                                ap=[[0, 64], [1, G]]),
                )
                nc.vector.tensor_mul(
                    out=oT_t[t][base:base + D, :], in0=av[0:64, :], in1=bc[:],
                )

        # ---------------- phase 3: output projection ----------------
        with tc.tile_pool(name="ph3_st", bufs=3) as stpool, \
             tc.tile_pool(name="ph3_ps", bufs=2, space="PSUM") as ps3:
            CT = O_LOC // 128
            for gc in range(GC):
                po = ps3.tile([128, C], F32, name="po", tag="po")
                for z in range(C // 512):
                    for ct in range(CT):
                        nc.tensor.matmul(
                            po[:, z * 512:(z + 1) * 512],
                            oT_t[ct][:, gc * 128:(gc + 1) * 128],
                            wp_sb[:, ct, z * 512:(z + 1) * 512],
                            start=(ct == 0), stop=(ct == CT - 1),
                        )
                st = stpool.tile([128, C], F32, name="st", tag="st")
                nc.scalar.copy(out=st[:], in_=po[:])
                nc.sync.dma_start(out=out_p[gc * 128:(gc + 1) * 128, :], in_=st[:])

    split_multi_waits(nc)
    return nc


_CACHE = {}


def make_in_maps(x, Wq, Wk, Wv, Wp):
    WqT, WkT, WvT, WpT = Wq.T, Wk.T, Wv.T, Wp.T
    in_maps = []
    for core in range(N_CORES):
        b, s = core // 2, core % 2
        osl = slice(s * O_LOC, (s + 1) * O_LOC)
        in_maps.append({
            "xT": np.ascontiguousarray(x[b].T),
            "wqT": np.ascontiguousarray(WqT[:, osl]),
            "wkT": np.ascontiguousarray(WkT[:, osl]),
            "wvT": np.ascontiguousarray(WvT[:, osl]),
            "wpT": np.ascontiguousarray(WpT[osl, :]),
        })
    return in_maps


def kernel(x, Wq, Wk, Wv, Wp, bp):
    x = np.ascontiguousarray(np.asarray(x, dtype=np.float32))
    in_maps = make_in_maps(x, np.asarray(Wq), np.asarray(Wk), np.asarray(Wv),
                           np.asarray(Wp))
    if "nc" not in _CACHE:
        _CACHE["nc"] = build_program()
    res = run_bass_kernel_spmd(_CACHE["nc"], in_maps, list(range(N_CORES)))
    out = np.zeros((B, G, C), np.float32)
    bp = np.asarray(bp, dtype=np.float32)
    for b in range(B):
        out[b] = res.results[2 * b]["out_p"] + res.results[2 * b + 1]["out_p"] + bp
    return out



# revision 41
# speedup vs baseline: 129.9776x; 129.9776x over previous
"""Multi-head attention (B=4, G=2048, C=1024, H=16) on 8 TRN2 NeuronCores.

Sharding: (batch x head-half). Core c handles batch c//2 and an 8-head
slice (c%2). Each core computes its heads' q/k/v projections, full
softmax attention, and a partial output projection over its 512
channels; the host sums core pairs and adds the bias.

Single-pass schedule tuned for PE continuity (the TRN2 PE p-state only
reaches 2.4 GHz while the engine stays busy):
  - all matmul operands stored bf16 (1 cycle/row, same as f32r, but
    half the SBUF/DMA footprint); PSUM accumulation stays fp32.
  - one persistent PSUM pool for the whole program (tags work x2 +
    av x2 = 8 banks) so there are no pool-transition barriers.
  - attention runs window-major ([65, 1024] av accumulators,
    double-buffered) with the AV matmul lagging scores by one key
    block; softmax denominators come free from a ones-column in v.
  - the denominator pipeline (DVE reciprocal straight off PSUM ->
    GpSimd partition_broadcast -> DVE multiply into bf16 oT) runs
    entirely off the PE critical path.
  - q/k/v projection chains and the output projection are injected
    into the attention instruction stream so the PE never idles while
    the scalar engine (exp, the second-longest engine at ~260us) keeps
    its backlog drained.
"""

from contextlib import ExitStack

import numpy as np
import ml_dtypes

import concourse.bass as bass
import concourse.tile as tile
from concourse import mybir
from concourse.bass_utils import run_bass_kernel_spmd
from concourse.vector_clock import ScopedClock, VectorClock
from concourse.tile_sem_assignment import N_PROCS

F32 = mybir.dt.float32
BF16 = mybir.dt.bfloat16
NP_BF16 = ml_dtypes.bfloat16

B, G, C, H = 4, 2048, 1024, 16
N_CORES = 8
H_LOC = H // 2          # 8 heads per core
O_LOC = H_LOC * 64      # 512 local channels
D = 64
CC = C // 128           # 8 contraction blocks for the projections
KC = G // 128           # 16 key blocks
W = 1024                # query window
NW = G // W             # 2 windows
NT = O_LOC // 128       # 4 head-pair tiles
SCALE = D ** -0.5
INJECT_PO = True


class SplitDrainTileContext(tile.TileContext):
    """Tail drain limited to one sync wait per instruction.

    This environment's walrus rejects >1 sync wait per instruction, so
    wait on each outstanding proc tick with its own NOP first and emit
    the drain bare.
    """

    def _drain_and_barrier(self, tick_clock, wait_clock):
        g = tick_clock.global_clock
        for p in range(N_PROCS):
            if g[p] > 0:
                nop = self.nc.sync.nop(nofuse=True)
                partial = VectorClock([g[q] if q == p else 0 for q in range(N_PROCS)])
                wait_clock.add_sem_waits(nop.ins, ScopedClock({None: partial}))
        self.nc.sync.drain()
        self.nc.all_engine_barrier()
        assert self.sems is not None
        popped = self.nc._tile_sem_poison_stack.pop()
        assert popped is self._sem_poison
        self.nc.clear_and_free_semaphores(list(self.sems.allocated().values()))
        self.nc.all_engine_barrier()


def split_multi_waits(nc):
    """Hoist extra sync waits onto NOPs before each offending instruction
    (this walrus accepts at most one sync wait per instruction)."""
    n_split = 0
    for f in nc.m.functions:
        for bb in f.blocks:
            insts = bb.instructions
            out = []
            for inst in insts:
                si = inst.sync_info
                waits = list(si.on_wait) if si and si.on_wait else []
                if len(waits) > 1:
                    for w in waits[:-1]:
                        nop = mybir.InstNoOp(
                            name=f"{inst.name}_w{n_split}",
                            engine=inst.engine,
                            ins=[],
                            outs=[],
                            sync_info=mybir.SyncInfo(on_wait=[w], on_update=[]),
                        )
                        out.append(nop)
                        n_split += 1
                    inst.sync_info = mybir.SyncInfo(
                        on_wait=[waits[-1]],
                        on_update=list(si.on_update) if si.on_update else [],
                    )
                out.append(inst)
            if len(out) != len(insts):
                bb.instructions[:] = out
    return n_split


def build_program():
    nc = bass.Bass()
    xT = nc.declare_dram_parameter("xT", [C, G], BF16, isOutput=False)
    wqT = nc.declare_dram_parameter("wqT", [C, O_LOC], BF16, isOutput=False)
    wkT = nc.declare_dram_parameter("wkT", [C, O_LOC], BF16, isOutput=False)
    wvT = nc.declare_dram_parameter("wvT", [C, O_LOC], BF16, isOutput=False)
    wpT = nc.declare_dram_parameter("wpT", [O_LOC, C], BF16, isOutput=False)
    out_p = nc.declare_dram_parameter("out_p", [G, C], F32, isOutput=True)

    rcp_dram = nc.dram_tensor("rcp_scratch", [NW * H_LOC, W], F32)

    with SplitDrainTileContext(nc) as tc, ExitStack() as ctx:
        pers = ctx.enter_context(tc.tile_pool(name="pers", bufs=1))
        dyn = ctx.enter_context(tc.tile_pool(name="dyn", bufs=1))
        ps = ctx.enter_context(tc.tile_pool(name="ps", bufs=2, space="PSUM"))

        wq_sb = pers.tile([128, CC, O_LOC], BF16, name="wq_sb", tag="wq")
        wk_sb = pers.tile([128, CC, O_LOC], BF16, name="wk_sb", tag="wk")
        wv_sb = pers.tile([128, CC, O_LOC], BF16, name="wv_sb", tag="wv")
        wp_sb = pers.tile([128, NT, C], BF16, name="wp_sb", tag="wp")
        x_sb = pers.tile([128, CC, G], BF16, name="x_sb", tag="x")
        qT = [pers.tile([128, G], BF16, name=f"qT{t}", tag=f"qT{t}") for t in range(NT)]
        kT = [pers.tile([128, G], BF16, name=f"kT{t}", tag=f"kT{t}") for t in range(NT)]
        v_sb = pers.tile([128, KC, H_LOC, 65], BF16, name="v_sb", tag="v")
        oT = [pers.tile([128, G], BF16, name=f"oT{t}", tag=f"oT{t}") for t in range(NT)]

        # Warm the activation table (Exp) before any real dependency.
        wa = dyn.tile([1, 2], F32, name="wa", tag="wa")
        nc.gpsimd.memset(wa[:], 0.0)
        wb = dyn.tile([1, 2], F32, name="wb", tag="wb")
        nc.scalar.activation(out=wb[:], in_=wa[:],
                             func=mybir.ActivationFunctionType.Exp, scale=1.0)

        # Input DMAs, split across the two HWDGE queues (SP + the idle
        # Activation engine) so the first projection chain unblocks ASAP.
        nc.scalar.dma_start(out=x_sb[:, 0:4, 0:W],
                            in_=xT[0:512, 0:W].rearrange("(cc p) g -> p cc g", p=128))
        nc.sync.dma_start(out=wk_sb[:], in_=wkT.rearrange("(cc p) o -> p cc o", p=128))
        nc.sync.dma_start(out=x_sb[:, 4:8, 0:W],
                          in_=xT[512:C, 0:W].rearrange("(cc p) g -> p cc g", p=128))
        nc.scalar.dma_start(out=wq_sb[:], in_=wqT.rearrange("(cc p) o -> p cc o", p=128))
        nc.sync.dma_start(out=wv_sb[:], in_=wvT.rearrange("(cc p) o -> p cc o", p=128))
        nc.scalar.dma_start(out=x_sb[:, :, W:G],
                            in_=xT[:, W:G].rearrange("(cc p) g -> p cc g", p=128))
        nc.sync.dma_start(out=wp_sb[:], in_=wpT.rearrange("(ct p) o -> p ct o", p=128))
        nc.gpsimd.memset(v_sb[:, :, :, 64:65], 1.0)

        def emit_qk_chain(dst, w_sb, t, win, cc_order=None):
            ccs = list(cc_order) if cc_order is not None else list(range(CC))
            pst = ps.tile([128, W], F32, name="pqk", tag="work")
            for z in range(2):
                for i, cc in enumerate(ccs):
                    nc.tensor.matmul(
                        pst[:, z * 512:(z + 1) * 512],
                        w_sb[:, cc, t * 128:(t + 1) * 128],
                        x_sb[:, cc, win * W + z * 512: win * W + (z + 1) * 512],
                        start=(i == 0), stop=(i == CC - 1),
                    )
            nc.vector.tensor_copy(out=dst[t][:, win * W:(win + 1) * W], in_=pst[:])

        def emit_v_chain(gc):
            pst = ps.tile([128, O_LOC], F32, name="pv", tag="work")
            for cc in range(CC):
                nc.tensor.matmul(
                    pst[:], x_sb[:, cc, gc * 128:(gc + 1) * 128], wv_sb[:, cc, :],
                    start=(cc == 0), stop=(cc == CC - 1),
                )
            nc.vector.tensor_copy(out=v_sb[:, gc, :, 0:64], in_=pst[:])

        def emit_po_partial(gc):
            """First 3 head-pair contractions of an output-projection
            chain — independent of the final window's oT. Returns the
            pst tile for finish."""
            pst = ps.tile([128, C], F32, name="ppo", tag="work")
            for ct in range(NT - 1):
                for z in range(2):
                    nc.tensor.matmul(
                        pst[:, z * 512:(z + 1) * 512],
                        oT[ct][:, gc * 128:(gc + 1) * 128],
                        wp_sb[:, ct, z * 512:(z + 1) * 512],
                        start=(ct == 0), stop=False,
                    )
            return pst

        def emit_po_finish(gc, pst, copy_on_act=False):
            ct = NT - 1
            for z in range(2):
                nc.tensor.matmul(
                    pst[:, z * 512:(z + 1) * 512],
                    oT[ct][:, gc * 128:(gc + 1) * 128],
                    wp_sb[:, ct, z * 512:(z + 1) * 512],
                    start=False, stop=True,
                )
            st = dyn.tile([128, C], F32, name="st", tag="st", bufs=2)
            if copy_on_act:
                nc.scalar.copy(out=st[:], in_=pst[:])
            else:
                nc.vector.tensor_copy(out=st[:], in_=pst[:])
            nc.sync.dma_start(out=out_p[gc * 128:(gc + 1) * 128, :], in_=st[:])

        def emit_po_chain(gc, copy_on_act=False):
            pst = emit_po_partial(gc)
            emit_po_finish(gc, pst, copy_on_act=copy_on_act)

        def emit_attn(win, h, inject, pending_fin):
            t, base = h // 2, (h % 2) * D
            av = ps.tile([65, W], F32, name="av", tag="av")
            kt, qt = kT[t], qT[t]
            exs = {}

            def emit_av(kc):
                ex = exs.pop(kc)
                for z in range(2):
                    nc.tensor.matmul(
                        av[:, z * 512:(z + 1) * 512],
                        v_sb[:, kc, h, :],
                        ex[:, z * 512:(z + 1) * 512],
                        start=(kc == 0), stop=(kc == KC - 1),
                    )

            for kc in range(KC):
                sc = ps.tile([128, W], F32, name="sc", tag="work")
                for z in range(2):
                    nc.tensor.matmul(
                        sc[:, z * 512:(z + 1) * 512],
                        kt[base:base + D, kc * 128:(kc + 1) * 128],
                        qt[base:base + D, win * W + z * 512: win * W + (z + 1) * 512],
                        start=True, stop=True,
                    )
                ex = dyn.tile([128, W], BF16, name="ex", tag="ex", bufs=4)
                nc.scalar.activation(out=ex[:], in_=sc[:],
                                     func=mybir.ActivationFunctionType.Exp,
                                     scale=SCALE)
                exs[kc] = ex
                if kc == 7 and pending_fin is not None:
                    pending_fin()
                for fn in inject.get(kc, ()):
                    fn()
                if kc >= 1:
                    emit_av(kc - 1)
            emit_av(KC - 1)

            # Denominator: spread the ones-row across all 128 partitions
            # (single-partition DVE ops run one lane — 40x slower), take
            # the reciprocal wide, then broadcast via a DRAM round-trip
            # (SBUF sources cannot take a 0-stride partition AP). The
            # whole chain runs on DVE + DMA — the PE never waits on it.
            # Only the final oT multiply is deferred into the NEXT
            # window's DVE stream so it cannot head-of-line-block the
            # injected projection casts there.
            den_row = dyn.tile([1, W], F32, name="den_row", tag="den_row", bufs=2)
            nc.vector.tensor_copy(out=den_row[:], in_=av[64:65, :])
            den_w = dyn.tile([128, W // 128], F32, name="den_w", tag="den_w", bufs=2)
            nc.sync.dma_start(out=den_w[:], in_=den_row[:])
            nc.vector.reciprocal(out=den_w[:], in_=den_w[:])
            wid = win * H_LOC + h
            nc.sync.dma_start(out=rcp_dram[wid, :], in_=den_w[:])
            bc = dyn.tile([64, W], F32, name="bc", tag="bc", bufs=2)
            row = rcp_dram[wid, :]
            nc.sync.dma_start(
                out=bc[:],
                in_=bass.AP(tensor=row.tensor, offset=row.offset,
                            ap=[[0, 64], [1, W]]),
            )

            def finisher():
                nc.vector.tensor_mul(
                    out=oT[t][base:base + D, win * W:(win + 1) * W],
                    in0=av[0:D, :], in1=bc[:],
                )

            return finisher

        def k_chain(t, win):
            return lambda: emit_qk_chain(kT, wk_sb, t, win)

        def q_chain(t, win):
            return lambda: emit_qk_chain(qT, wq_sb, t, win)

        def v_chain(gc):
            return lambda: emit_v_chain(gc)

        def po_chain(gc):
            return lambda: emit_po_chain(gc)

        # ---- schedule ----
        # Scores for ANY query window read the FULL kT[t] (keys span all
        # of G), so both k-chain windows of a tile must land before head
        # 2t's attention starts. Only qT windows defer per-window.
        # The very first chain leads with the LAST-arriving x chunk so the
        # PE's first matmul only issues once all its inputs are resident —
        # an early mid-chain stall can pin the PE p-state low for the
        # whole run.
        emit_qk_chain(kT, wk_sb, 0, 0, cc_order=[7, 6, 5, 4, 3, 2, 1, 0])
        emit_qk_chain(qT, wq_sb, 0, 0)
        emit_v_chain(0)
        emit_v_chain(1)

        # A head-pair tile t serves heads 2t and 2t+1, so each pair of
        # attention windows needs just one new k window (split between the
        # even/odd head) and one q window. Injections are spread so no
        # window's PE load overshoots the Act pace (16 exps) by much.
        inj = {
            # v blocks arrive just ahead of their first use in (win0, h0);
            # the second kT window lands before sc(kc=8) needs it.
            (0, 0): {kc: ([v_chain(kc + 2)] + ([k_chain(0, 1)] if kc == 2 else []))
                     for kc in range(0, 14)},
            (0, 1): {2: [k_chain(1, 0)], 9: [q_chain(1, 0)]},
            (0, 2): {1: [k_chain(1, 1)]},
            (0, 3): {2: [k_chain(2, 0)], 9: [q_chain(2, 0)]},
            (0, 4): {1: [k_chain(2, 1)]},
            (0, 5): {2: [k_chain(3, 0)], 9: [q_chain(3, 0)]},
            (0, 6): {1: [k_chain(3, 1)]},
            (0, 7): {2: [q_chain(0, 1)]},
            (1, 1): {2: [q_chain(1, 1)]},
            (1, 3): {2: [q_chain(2, 1)]},
            (1, 5): {2: [q_chain(3, 1)]},
        }
        if INJECT_PO:
            # window-0 output projection interleaves into window-1
            # attention (merged per-kc: do NOT clobber the q chains)
            for (wh, kc, fn) in [
                ((1, 2), 4, po_chain(0)), ((1, 2), 10, po_chain(1)),
                ((1, 3), 9, po_chain(2)),
                ((1, 4), 4, po_chain(3)), ((1, 4), 10, po_chain(4)),
                ((1, 5), 9, po_chain(5)),
                ((1, 6), 4, po_chain(6)), ((1, 6), 10, po_chain(7)),
            ]:
                inj.setdefault(wh, {}).setdefault(kc, []).append(fn)
        fin = None
        for win in range(NW):
            for h in range(H_LOC):
                fin = emit_attn(win, h, inj.get((win, h), {}), fin)
        fin()
        # Tail: software-pipeline the remaining output-projection chains —
        # each chain's first 3 contractions are independent of the last
        # window's oT, so they overlap the final denominator latency.
        pending = []
        for gc in (range(8, 16) if INJECT_PO else range(16)):
            pending.append((gc, emit_po_partial(gc)))
            if len(pending) == 2:
                g0, p0 = pending.pop(0)
                emit_po_finish(g0, p0, copy_on_act=True)
        for g0, p0 in pending:
            emit_po_finish(g0, p0, copy_on_act=True)

    split_multi_waits(nc)
    return nc


_CACHE = {}


def make_in_maps(x, Wq, Wk, Wv, Wp):
    x = np.asarray(x, dtype=np.float32)
    WqT = np.asarray(Wq, dtype=np.float32).T
    WkT = np.asarray(Wk, dtype=np.float32).T
    WvT = np.asarray(Wv, dtype=np.float32).T
    WpT = np.asarray(Wp, dtype=np.float32).T
    in_maps = []
    for core in range(N_CORES):
        b, s = core // 2, core % 2
        osl = slice(s * O_LOC, (s + 1) * O_LOC)
        in_maps.append({
            "xT": np.ascontiguousarray(x[b].T).astype(NP_BF16),
            "wqT": np.ascontiguousarray(WqT[:, osl]).astype(NP_BF16),
            "wkT": np.ascontiguousarray(WkT[:, osl]).astype(NP_BF16),
            "wvT": np.ascontiguousarray(WvT[:, osl]).astype(NP_BF16),
            "wpT": np.ascontiguousarray(WpT[osl, :]).astype(NP_BF16),
        })
    return in_maps


def kernel(x, Wq, Wk, Wv, Wp, bp):
    in_maps = make_in_maps(x, Wq, Wk, Wv, Wp)
    if "nc" not in _CACHE:
        _CACHE["nc"] = build_program()
    res = run_bass_kernel_spmd(_CACHE["nc"], in_maps, list(range(N_CORES)))
    out = np.zeros((B, G, C), np.float32)
    bp = np.asarray(bp, dtype=np.float32)
    for b in range(B):
        out[b] = res.results[2 * b]["out_p"] + res.results[2 * b + 1]["out_p"] + bp
    return out
